# revision 2
# baseline (speedup 1.0000x reference)
"""Trainium2 Bass kernel for nn_MultiHeadQKVAttention_41936060678242.

Permuted-sequence layout: seq s <-> (p, t) with s = 16p + t (p partition,
t tile index). Every DRAM access becomes per-partition contiguous (16KB
runs for Q/K/out, 2KB for V/w), slashing DMA descriptor count ~10x vs the
(t p) tiling. The permutation is self-consistent end-to-end:
  * "n-tile t" holds keys {16p+t}; its S rows, exp mask rows, and V rows
    (v_stage[:, t, :]) all carry the same permutation, and softmax sums
    over all n regardless of order.
  * "m-tile t" holds queries {16p+t}; the per-m denominator, q mask, and
    output rows follow it; the out store slot (p, t, :) -> DRAM row 16p+t
    is again contiguous per partition.

Math (per batch b): routing = Q@K^T masked -> softmax -> @V -> tile(8) ->
conv1d(k=1). tile+conv collapse to head @ w_eff with
w_eff[v, d] = sum_h w_o[d, 32h+v]; q_pres/inv_scale fold into Q rows;
k_pres becomes the additive -C_BIG exp bias; softmax max-shift replaced
by constant C_SHIFT (logits bounded).

Scores computed transposed S_T[n, m] so the key mask is a per-partition
ACT bias and P_T feeds P@V directly; denominator from a ones-column in V.
P@V for n-tile i is emitted during n-tile i+1 (software pipeline).

Sharding: data-parallel over batch B=8 across the 8 NeuronCores.
"""

import numpy as np

import concourse.bass as bass
import concourse.mybir as mybir
import concourse.tile as tile
from concourse import bacc, bass_utils
from concourse.bass import ds, ts
from concourse.masks import make_identity

F32 = mybir.dt.float32
F16 = mybir.dt.float16

N_CORES = 8
B, M, N, D, V = 8, 2048, 2048, 256, 32
P = 128
NT = N // P            # 16 key tiles
MT = M // P            # 16 query tiles
DH = D // P            # 2 contraction halves
MBS = 1024             # m-block (free dim) per S-psum tile
NMB = M // MBS         # 2
CH = MBS // 512        # 2 chunks of 512 per m-block (psum-bank limit)
KG = 4                 # K-tile prep group size

INV_SCALE = float(1.0 / np.sqrt(np.float32(32.0)))
C_BIG = float(np.float32(1e32) * np.float32(INV_SCALE))  # mask magnitude, pre-scaled
C_SHIFT = 8.0          # global exp shift (softmax-invariant), keeps exp in fp16 range

_NC_CACHE = {}


class _Ctx:
    pass


def _pe_transpose(nc, cx, psum_s, dst, src):
    pt = psum_s.tile([P, P], F16, tag="s")
    nc.tensor.matmul(pt, src, cx.ident, is_transpose=True, start=True, stop=True)
    if cx.tp_count % 2 == 0:
        nc.vector.tensor_copy(out=dst, in_=pt)
    else:
        nc.scalar.copy(out=dst, in_=pt)
    cx.tp_count += 1


def _emit_prep(nc, cx, singles, stage, psum_s, aps):
    """Everything the attention loop needs up front: masks, V, Q^T, w staged."""
    q_d, k_d, v_d, qp_d, kp_d, w_d, b_d, out_d = aps
    cx.tp_count = 0

    # ACT exp-table preload (overlaps with DMA prep)
    dummy = singles.tile([P, 1], F32)
    nc.vector.memset(dummy, 0.0)
    nc.scalar.activation(dummy, dummy, mybir.ActivationFunctionType.Exp)

    # presence masks, p-outer: qp_sb[p, t] = q_pres[16p+t]
    qp_sb = singles.tile([P, MT], F32)
    kp_sb = singles.tile([P, NT], F32)
    nc.sync.dma_start(out=qp_sb, in_=qp_d.rearrange("(p t) -> p t", p=P))
    nc.sync.dma_start(out=kp_sb, in_=kp_d.rearrange("(p t) -> p t", p=P))
    qscale = singles.tile([P, MT], F32)
    nc.vector.tensor_scalar_mul(qscale, qp_sb, INV_SCALE)
    # neg[p, t] = kp*C_BIG - C_BIG (exactly 0 when kp==1), then -C_SHIFT.
    # Two steps: C_BIG + C_SHIFT rounds to C_BIG in f32, so a fused constant
    # would silently drop the shift.
    cx.neg_sb = singles.tile([P, NT], F32)
    nc.vector.tensor_scalar(cx.neg_sb, kp_sb, C_BIG, -C_BIG,
                            mybir.AluOpType.mult, mybir.AluOpType.add)
    nc.vector.tensor_scalar_add(cx.neg_sb, cx.neg_sb, -C_SHIFT)

    # identity (fp16) for PE transposes
    cx.ident = singles.tile([P, P], F16)
    make_identity(nc, cx.ident)

    # V (+ ones column for the softmax denominator), p-outer: 2KB/partition
    v_stage = singles.tile([P, NT, V], F32)
    nc.gpsimd.dma_start(out=v_stage, in_=v_d.rearrange("(p t) v -> p t v", p=P))
    cx.v_aug = singles.tile([P, NT, V + 1], F16)
    nc.vector.tensor_copy(out=cx.v_aug[:, :, 0:V], in_=v_stage)
    nc.vector.memset(cx.v_aug[:, :, V:V + 1], 1.0)

    # w staged early (p-outer: w_f16[p, j, c] = w_o[2p+j, c]); b broadcast via PE
    cx.w_f16 = singles.tile([P, DH, D], F16)
    w_stage = singles.tile([P, DH, D], F32)
    nc.gpsimd.dma_start(out=w_stage, in_=w_d.rearrange("(p j) c -> p j c", p=P))
    nc.vector.tensor_copy(out=cx.w_f16, in_=w_stage)
    b_row = singles.tile([1, D], F32)
    nc.gpsimd.dma_start(out=b_row, in_=b_d.rearrange("(o d) -> o d", o=1))
    cx.b_row = b_row

    # Q: load -> scale by qp*inv_scale (casts to fp16) -> PE transpose into QT
    cx.qt = singles.tile([P, DH, M], F16)
    q_tiled = q_d.rearrange("(p t) d -> p t d", p=P)
    for g in range(4):
        q_stage = stage.tile([P, 4, D], F32, tag="st")
        eng = nc.sync if g % 2 == 0 else nc.scalar
        eng.dma_start(out=q_stage, in_=q_tiled[:, ts(g, 4), :])
        q_f16 = stage.tile([P, 4, D], F16, tag="qf16")
        for j in range(4):
            t = 4 * g + j
            nc.vector.tensor_scalar_mul(q_f16[:, j, :], q_stage[:, j, :],
                                        qscale[:, t:t + 1])
        for j in range(4):
            for dh in range(DH):
                _pe_transpose(nc, cx, psum_s,
                              cx.qt[:, dh, ts(4 * g + j, P)],
                              q_f16[:, j, ts(dh, P)])

    cx.kt = singles.tile([P, DH, N], F16)
    cx.k_tiled = k_d.rearrange("(p t) d -> p t d", p=P)


def _emit_kgroup(nc, cx, stage, psum_s, g):
    """Load + cast + transpose K tiles [4g, 4g+4) into KT."""
    k_stage = stage.tile([P, KG, D], F32, tag="st")
    eng = nc.sync if g % 2 == 0 else nc.scalar
    eng.dma_start(out=k_stage, in_=cx.k_tiled[:, ts(g, KG), :])
    k_f16 = stage.tile([P, KG, D], F16, tag="f16")
    nc.vector.tensor_copy(out=k_f16, in_=k_stage)
    for j in range(KG):
        for dh in range(DH):
            _pe_transpose(nc, cx, psum_s,
                          cx.kt[:, dh, ts(KG * g + j, P)],
                          k_f16[:, j, ts(dh, P)])


def _emit_main(nc, cx, stage, exps, psum_s, psum_n):
    """S_T = K'Q'^T -> exp -> num/den accumulate.  P@V software-pipelined;
    K-tile prep interleaved one group ahead."""
    num_ps = psum_n.tile([V + 1, M], F32)
    cx.num_ps = num_ps

    def emit_num(pnt, pmb, pexp):
        for ch in range(CH):
            nc.tensor.matmul(
                num_ps[:, ds(pmb * MBS + ch * 512, 512)],
                cx.v_aug[:, pnt, :],
                pexp[:, ts(ch, 512)],
                start=(pnt == 0), stop=(pnt == NT - 1))

    pending = []
    for nt in range(NT):
        if nt % KG == 0:
            _emit_kgroup(nc, cx, stage, psum_s, nt // KG)
        new_pending = []
        for mb in range(NMB):
            s_ps = psum_s.tile([P, MBS], F32, tag="s")
            for dh in range(DH):
                for ch in range(CH):
                    nc.tensor.matmul(
                        s_ps[:, ts(ch, 512)],
                        cx.kt[:, dh, ts(nt, P)],
                        cx.qt[:, dh, ds(mb * MBS + ch * 512, 512)],
                        start=(dh == 0), stop=(dh == DH - 1))
            exp_t = exps.tile([P, MBS], F16)
            nc.scalar.activation(exp_t, s_ps,
                                 mybir.ActivationFunctionType.Exp,
                                 bias=cx.neg_sb[:, nt:nt + 1], scale=1.0)
            new_pending.append((nt, mb, exp_t))
        for args in pending:
            emit_num(*args)
        pending = new_pending
    for args in pending:
        emit_num(*args)


def _emit_wprep(nc, cx, singles, psum_s):
    """weff[v, d] with d = 2p+j: column-reduce w over heads, then transpose.

    w_f16[p, j, c] = w_o[2p+j, c].  wsum[p, j, v] = sum_h w_f16[p, j, 32h+v].
    PE-transpose wsum[:, j, :] -> wT[v, j, p]; the matmul moving operand
    reads wT with free dims (p, j) giving natural d = 2p+j order.
    """
    wv = cx.w_f16.rearrange("p j (h v) -> p j h v", v=V)
    wsum4 = singles.tile([P, DH, 4, V], F16)
    nc.vector.tensor_add(wsum4, wv[:, :, 0:4, :], wv[:, :, 4:8, :])
    wsum2 = singles.tile([P, DH, 2, V], F16)
    nc.vector.tensor_add(wsum2, wsum4[:, :, 0:2, :], wsum4[:, :, 2:4, :])
    wsum = singles.tile([P, DH, V], F16)
    nc.vector.tensor_add(wsum, wsum2[:, :, 0, :], wsum2[:, :, 1, :])
    wT = singles.tile([V, DH, P], F16)
    for j in range(DH):
        pt = psum_s.tile([P, P], F16, tag="s")
        nc.tensor.matmul(pt[0:V, :], wsum[:, j, :], cx.ident,
                         is_transpose=True, start=True, stop=True)
        nc.scalar.copy(out=wT[:, j, :], in_=pt[0:V, :])
    # weff[v, d] view with free dims (p, j): d = 2p + j
    cx.weff = wT.rearrange("v j p -> v p j")

    # b broadcast to all partitions: ones[128,1] (x) b_row[1,256] via PE
    onecol = singles.tile([1, P], F32)
    nc.vector.memset(onecol, 1.0)
    b_ps = psum_s.tile([P, D], F32, tag="s")
    nc.tensor.matmul(b_ps, onecol, cx.b_row, start=True, stop=True)
    cx.b_bcast = singles.tile([P, D], F32)
    nc.vector.tensor_copy(out=cx.b_bcast, in_=b_ps)


def _emit_epilogue(nc, cx, singles, outs_pool, psum_s, out_d):
    num_ps = cx.num_ps
    num_f16 = singles.tile([V, M], F16)
    nc.scalar.copy(out=num_f16, in_=num_ps[0:V, :])
    den_sb = singles.tile([V + 1, M], F16)  # only row V used
    nc.vector.tensor_copy(out=den_sb[V:V + 1, 0:M // 2],
                          in_=num_ps[V:V + 1, 0:M // 2])
    nc.scalar.copy(out=den_sb[V:V + 1, M // 2:M], in_=num_ps[V:V + 1, M // 2:M])
    ones1 = singles.tile([V + 1, 1], F16)
    nc.vector.memset(ones1[V:V + 1, :], 1.0)
    denT_ps = psum_s.tile([P, MT], F32, tag="s")
    for mt in range(MT):
        nc.tensor.matmul(denT_ps[:, mt:mt + 1],
                         den_sb[V:V + 1, ts(mt, P)],
                         ones1[V:V + 1, :], start=True, stop=True)
    recipT = singles.tile([P, MT], F32)
    nc.vector.reciprocal(recipT, denT_ps)

    o_stage = outs_pool.tile([P, MT, D], F32)
    for mt in range(MT):
        o_ps = psum_s.tile([P, D], F32, tag="s")
        nc.tensor.matmul(o_ps, num_f16[:, ts(mt, P)], cx.weff,
                         start=True, stop=True)
        nc.vector.scalar_tensor_tensor(
            out=o_stage[:, mt, :], in0=o_ps, scalar=recipT[:, mt:mt + 1],
            in1=cx.b_bcast, op0=mybir.AluOpType.mult,
            op1=mybir.AluOpType.add)
    # one p-outer store: 16KB contiguous per partition
    nc.sync.dma_start(out=out_d.rearrange("(p t) d -> p t d", p=P), in_=o_stage)


def _build_nc(reps=1, ablate="full"):
    key = ("nc", reps, ablate)
    if key in _NC_CACHE:
        return _NC_CACHE[key]

    nc = bacc.Bacc("TRN2", target_bir_lowering=False, debug=False,
                   num_devices=N_CORES)

    q_d = nc.dram_tensor("queries", [M, D], F32, kind="ExternalInput").ap()
    k_d = nc.dram_tensor("keys", [N, D], F32, kind="ExternalInput").ap()
    v_d = nc.dram_tensor("values", [N, V], F32, kind="ExternalInput").ap()
    qp_d = nc.dram_tensor("q_pres", [M], F32, kind="ExternalInput").ap()
    kp_d = nc.dram_tensor("k_pres", [N], F32, kind="ExternalInput").ap()
    w_d = nc.dram_tensor("w_o", [D, D], F32, kind="ExternalInput").ap()
    b_d = nc.dram_tensor("b_o", [D], F32, kind="ExternalInput").ap()
    out_d = nc.dram_tensor("out", [M, D], F32, kind="ExternalOutput").ap()
    aps = (q_d, k_d, v_d, qp_d, kp_d, w_d, b_d, out_d)

    with tile.TileContext(nc) as tc:
        with (
            tc.tile_pool(name="singles", bufs=1) as singles,
            tc.tile_pool(name="stage", bufs=2) as stage,
            tc.tile_pool(name="exps", bufs=4) as exps,
            tc.tile_pool(name="outs", bufs=2) as outs_pool,
            tc.tile_pool(name="psum_s", bufs=2, space="PSUM") as psum_s,
            tc.tile_pool(name="psum_n", bufs=1, space="PSUM") as psum_n,
        ):
            cx = _Ctx()
            cx.dbg = {}
            _emit_prep(nc, cx, singles, stage, psum_s, aps)
            for r in range(reps):
                if r > 0:
                    _emit_prep(nc, cx, singles, stage, psum_s, aps)
                _emit_main(nc, cx, stage, exps, psum_s, psum_n)
                _emit_wprep(nc, cx, singles, psum_s)
                _emit_epilogue(nc, cx, singles, outs_pool, psum_s, out_d)

    nc.compile()
    _NC_CACHE["dbg"] = cx.dbg
    _NC_CACHE[key] = nc
    return nc


def _in_maps(queries, keys, values, q_pres, k_pres, w_o, b_o):
    f32 = np.float32
    return [
        {
            "queries": np.ascontiguousarray(queries[c], dtype=f32),
            "keys": np.ascontiguousarray(keys[c], dtype=f32),
            "values": np.ascontiguousarray(values[c], dtype=f32),
            "q_pres": np.ascontiguousarray(q_pres[c], dtype=f32),
            "k_pres": np.ascontiguousarray(k_pres[c], dtype=f32),
            "w_o": np.ascontiguousarray(w_o, dtype=f32),
            "b_o": np.ascontiguousarray(b_o, dtype=f32),
        }
        for c in range(N_CORES)
    ]


def kernel(queries, keys, values, q_pres, k_pres, w_o, b_o):
    nc = _build_nc()
    in_maps = _in_maps(queries, keys, values, q_pres, k_pres, w_o, b_o)
    res = bass_utils.run_bass_kernel_spmd(nc, in_maps, core_ids=list(range(N_CORES)))
    return np.stack([res.results[c]["out"] for c in range(N_CORES)]).astype(np.float32)
